# revision 1
# baseline (speedup 1.0000x reference)
"""Trainium2 Bass kernel for a 2-layer GCN (GCNConv -> ReLU -> GCNConv -> log_softmax).

Strategy (8 NeuronCores, SPMD, one NEFF):
  * Nodes are range-sharded across the 8 cores by destination. Each core owns
    N/8 destination nodes and all edges pointing at them.
  * GCN symmetric normalization factorizes: out = D^-1/2 (A+I) D^-1/2 h.
    Rows are pre-scaled by dinv once per node, aggregated unnormalized, and
    post-scaled by dinv. The self-loop term is added locally from an f32
    copy of the core's own rows (never gathered).
  * The gather h[src] uses dma_gather of 256-byte rows stored as
    [bf16(hi) | bf16(lo)] pairs; hi/lo splitting keeps ~fp32 precision while
    the segment-sum matmuls run at bf16 speed.
  * dma_gather descriptor generation is Q7-serial (~7.5 ns/idx) and a single
    instruction must stay <= 1024 indices (SWDGE ring limit), so gathers are
    emitted as sub-gathers of <= 1024 idxs writing disjoint slices of one
    SBUF tile.
  * Segment-sum: edges are grouped per (dst 128-node block, src bank); a
    one-hot S matrix [128 edges x 128 node slots] is built on the Vector
    engine (is_equal vs an iota row, batched) and the TensorEngine computes
    S^T @ msg accumulating into a PSUM tile per dst block.
  * dma_gather indices are int16 -> the gather source is split into 4 banks
    of 25000 rows.
  * One NEFF runs on all 8 cores: tiles-per-(block,bank) is fixed to the max
    over the 8 cores (pad slots use idx=0 and dst_rel=-1 -> zero S rows).
  * Between layers the per-core h1 slices are AllGathered (HBM collective).
"""

import os
import sys
import numpy as np

P = 128          # partitions / edge-tile size
FEAT = 64        # hidden feature width
OUTC = 32        # output classes
SG_BLOCKS = 6    # dst blocks per supergroup (psum pool is 6 + 2 spare banks)
MAX_BANK_ROWS = 25000   # int16 gather index limit
GMAX = 1024      # max idxs per dma_gather instruction (SWDGE ring limit)


class _PhaseDone(Exception):
    pass


class Schedule:
    pass


# --------------------------------------------------------------------------
# Host-side schedule construction
# --------------------------------------------------------------------------

def build_schedule(src, dst, n_nodes, n_cores):
    """64-slot-quantized runs per (block, bank); 128-edge tiles may span two
    adjacent runs (two matmul targets). Schedule is shared by all cores
    (SPMD): run lengths = 64*ceil(max-over-cores/64)."""
    assert n_nodes % n_cores == 0
    Q = 64
    nslice = n_nodes // n_cores
    nblk = (nslice + P - 1) // P
    nbank = (n_nodes + MAX_BANK_ROWS - 1) // MAX_BANK_ROWS
    bankrows = (n_nodes + nbank - 1) // nbank
    assert bankrows <= 32767

    src_a = src.astype(np.int64)
    dst_a = dst.astype(np.int64)
    core = dst_a // nslice
    block = (dst_a % nslice) // P
    bank = src_a // bankrows

    key = (core * nblk + block) * nbank + bank
    counts = np.bincount(key, minlength=n_cores * nblk * nbank).reshape(
        n_cores, nblk, nbank)
    R = (np.ceil(counts.max(axis=0) / Q) * Q).astype(np.int64)  # slots/cell
    for b in range(nblk):
        if R[b].sum() == 0:
            R[b, 0] = Q

    sgs = [list(range(sgi, min(sgi + SG_BLOCKS, nblk)))
           for sgi in range(0, nblk, SG_BLOCKS)]

    sch = Schedule()
    sch.n_nodes, sch.n_cores, sch.nslice = n_nodes, n_cores, nslice
    sch.nblk, sch.nbank, sch.bankrows = nblk, nbank, bankrows
    sch.sgs = sgs

    # per (sg, bank): run slot offsets, LR (total slots, padded to 128)
    sch.run_off = []      # [s_i][(blk,b)] -> slot offset within (sg,bank)
    sch.gather_L = []
    for s_i, blks in enumerate(sgs):
        offs = {}
        Ls = []
        for b_i in range(nbank):
            o = 0
            for blk in blks:
                offs[(blk, b_i)] = o
                o += int(R[blk, b_i])
            o = ((o + P - 1) // P) * P   # pad gather to 128-multiple
            Ls.append(o)
        sch.run_off.append(offs)
        sch.gather_L.append(Ls)
    sch.maxL = max(max(Ls) for Ls in sch.gather_L) if sgs else 0

    # matmul sequence per sg: block-major; for each block, for each bank,
    # the tiles overlapping its run (boundary tiles shared with neighbours).
    # Each entry: (bank, tile_col, dcol, blk, start, stop)
    sch.mmseq = []        # [s_i] -> list of entries
    ncols = 0
    dcol_map = []         # (s_i, bank, tile_col, blk) per dstrel column
    for s_i, blks in enumerate(sgs):
        seq = []
        per_block = {blk: [] for blk in blks}
        for blk in blks:
            for b_i in range(nbank):
                r0 = sch.run_off[s_i][(blk, b_i)]
                r1 = r0 + int(R[blk, b_i])
                if r1 == r0:
                    continue
                t0, t1 = r0 // P, (r1 - 1) // P
                for tc_ in range(t0, t1 + 1):
                    per_block[blk].append((b_i, tc_))
        for blk in blks:
            lst = per_block[blk]
            for i, (b_i, tc_) in enumerate(lst):
                seq.append([b_i, tc_, ncols, blk, i == 0, i == len(lst) - 1])
                dcol_map.append((s_i, b_i, tc_, blk))
                ncols += 1
        sch.mmseq.append(seq)
    sch.ncols = ncols
    sch.R = R

    # gidx layout
    off = 0
    gidx_off = {}
    for s_i in range(len(sgs)):
        for b_i in range(nbank):
            gidx_off[(s_i, b_i)] = off
            off += sch.gather_L[s_i][b_i] // 16
    sch.gidx_cols = off
    sch.gidx_off = gidx_off

    # ---------------- per-core arrays ----------------
    order = np.lexsort((bank, block, core))
    s_o, d_o = src_a[order], dst_a[order]
    grp_key = (core[order] * nblk + block[order]) * nbank + bank[order]
    uniq, starts = np.unique(grp_key, return_index=True)
    grp_start = {int(k): int(v) for k, v in zip(uniq, starts)}
    grp_count = {int(k): int(v) for k, v in
                 zip(uniq, np.diff(np.append(starts, len(grp_key))))}

    sch.core_gidx = []
    sch.core_dstrel = []
    for c in range(n_cores):
        gidx = np.zeros((16, sch.gidx_cols), dtype=np.int16)
        dstrel = np.full((P, sch.ncols), -1.0, dtype=np.float32)
        # edge slot data per (sg, bank)
        slot_src = {}
        slot_dst = {}
        for s_i, blks in enumerate(sgs):
            for b_i in range(nbank):
                L = sch.gather_L[s_i][b_i]
                if L == 0:
                    continue
                idx_lin = np.zeros(L, dtype=np.int16)
                dst_lin = np.full(L, -1, dtype=np.int64)
                for blk in blks:
                    k = int((c * nblk + blk) * nbank + b_i)
                    cnt = grp_count.get(k, 0)
                    if not cnt:
                        continue
                    st = grp_start.get(k, 0)
                    o = sch.run_off[s_i][(blk, b_i)]
                    idx_lin[o:o + cnt] = (
                        s_o[st:st + cnt] - b_i * bankrows).astype(np.int16)
                    dst_lin[o:o + cnt] = d_o[st:st + cnt]
                go = gidx_off[(s_i, b_i)]
                gidx[:, go:go + L // 16] = idx_lin.reshape(L // 16, 16).T
                slot_src[(s_i, b_i)] = idx_lin
                slot_dst[(s_i, b_i)] = dst_lin
        # dstrel per (tile, target-block) column
        for dcol, (s_i, b_i, tc_, blk) in enumerate(dcol_map):
            dl = slot_dst.get((s_i, b_i))
            if dl is None:
                continue
            seg = dl[tc_ * P:(tc_ + 1) * P]
            base = c * nslice + blk * P
            vals = seg - base
            vals = np.where((seg >= 0) & (vals >= 0) & (vals < P),
                            vals, -1).astype(np.float32)
            dstrel[:len(seg), dcol] = vals
        sch.core_gidx.append(np.tile(gidx, (8, 1)))
        sch.core_dstrel.append(dstrel)
    return sch


def numpy_check_schedule(sch, src, dst, n_nodes):
    """Emulate the device aggregation (no self loops) in numpy."""
    rng = np.random.default_rng(0)
    g = rng.standard_normal((n_nodes, FEAT)).astype(np.float32)
    ref = np.zeros((n_nodes, FEAT), np.float32)
    np.add.at(ref, dst, g[src])
    out = np.zeros((n_nodes, FEAT), np.float32)
    for c in range(sch.n_cores):
        gidx = sch.core_gidx[c]
        dstrel = sch.core_dstrel[c]
        msgs = {}
        for s_i in range(len(sch.sgs)):
            for b_i in range(sch.nbank):
                L = sch.gather_L[s_i][b_i]
                if L == 0:
                    continue
                go = sch.gidx_off[(s_i, b_i)]
                idx = gidx[:16, go:go + L // 16].T.reshape(-1)
                rows = g[b_i * sch.bankrows:
                         min((b_i + 1) * sch.bankrows, n_nodes)]
                msgs[(s_i, b_i)] = rows[idx]
        for s_i in range(len(sch.sgs)):
            for (b_i, tc_, dcol, blk, start, stop) in sch.mmseq[s_i]:
                m = msgs[(s_i, b_i)][tc_ * P:(tc_ + 1) * P]
                S = (dstrel[:, dcol][:, None] ==
                     np.arange(P)[None, :]).astype(np.float32)
                base = c * sch.nslice + blk * P
                hi = min(base + P, n_nodes)
                out[base:hi] += (S.T @ m)[:hi - base]
    return np.abs(out - ref).max() / (np.abs(ref).max() + 1e-9)


# --------------------------------------------------------------------------
# Bass program
# --------------------------------------------------------------------------

def build_program(sch, phases=5):
    import concourse.mybir as mybir
    import concourse.tile as tile
    from concourse import bacc
    from concourse.masks import make_identity

    dt = mybir.dt
    AF = mybir.ActivationFunctionType
    OP = mybir.AluOpType

    n_cores = sch.n_cores
    nslice, nblk, nbank = sch.nslice, sch.nblk, sch.nbank
    NT = sch.ncols
    subph = os.environ.get("GCN_SUBPH", "full")

    nc = bacc.Bacc("TRN2", target_bir_lowering=False, debug=False,
                   num_devices=n_cores)

    xT = nc.dram_tensor("xT", [FEAT, nslice], dt.float32, kind="ExternalInput")
    W1 = nc.dram_tensor("W1", [FEAT, FEAT], dt.float32, kind="ExternalInput")
    W2 = nc.dram_tensor("W2", [FEAT, OUTC], dt.float32, kind="ExternalInput")
    b1r = nc.dram_tensor("b1r", [P, FEAT], dt.float32, kind="ExternalInput")
    b2r = nc.dram_tensor("b2r", [P, OUTC], dt.float32, kind="ExternalInput")
    iota = nc.dram_tensor("iota", [P, P], dt.bfloat16, kind="ExternalInput")
    dinv = nc.dram_tensor("dinv", [P, nblk], dt.float32, kind="ExternalInput")
    gidx = nc.dram_tensor("gidx", [P, max(sch.gidx_cols, 16)], dt.int16,
                          kind="ExternalInput")
    dstrel = nc.dram_tensor("dstrel", [P, NT], dt.bfloat16,
                            kind="ExternalInput")
    z_out = nc.dram_tensor("z", [nslice, OUTC], dt.float32,
                           kind="ExternalOutput")

    g0_slice = nc.dram_tensor("g0_slice", [nslice, 2 * FEAT], dt.bfloat16)
    g0f = nc.dram_tensor("g0f", [nslice, FEAT], dt.float32)
    g1f = nc.dram_tensor("g1f", [nslice, FEAT], dt.float32)
    g0_full = nc.dram_tensor("g0_full", [sch.n_nodes, 2 * FEAT], dt.bfloat16,
                             addr_space="Shared")
    g1_slice = nc.dram_tensor("g1_slice", [nslice, 2 * FEAT], dt.bfloat16)
    g1_full = nc.dram_tensor("g1_full", [sch.n_nodes, 2 * FEAT], dt.bfloat16,
                             addr_space="Shared")

    replica_groups = [list(range(n_cores))]
    maxC = sch.maxL // P

    with tile.TileContext(nc) as tc:
        with (
            tc.tile_pool(name="const", bufs=1) as constp,
            tc.tile_pool(name="gather", bufs=nbank + 2) as gatherp,
            tc.tile_pool(name="sbuild", bufs=4) as sp,
            tc.tile_pool(name="gidxp", bufs=2 * (nbank + 2)) as gidxp,
            tc.tile_pool(name="epi", bufs=3) as epip,
            tc.tile_pool(name="hilo", bufs=3) as hilop,
            tc.tile_pool(name="psA", bufs=6, space="PSUM") as psA,
            tc.tile_pool(name="psT", bufs=1, space="PSUM") as psT,
            tc.tile_pool(name="psZ", bufs=1, space="PSUM") as psZ,
        ):
          try:
            W1_sb = constp.tile([FEAT, FEAT], dt.float32)
            nc.sync.dma_start(out=W1_sb[:], in_=W1.ap())
            W2_sb = constp.tile([FEAT, OUTC], dt.float32)
            nc.sync.dma_start(out=W2_sb[:], in_=W2.ap())
            b1_sb = constp.tile([P, FEAT], dt.float32)
            nc.sync.dma_start(out=b1_sb[:], in_=b1r.ap())
            b2_sb = constp.tile([P, OUTC], dt.float32)
            nc.sync.dma_start(out=b2_sb[:], in_=b2r.ap())
            iota_sb = constp.tile([P, P], dt.bfloat16)
            nc.sync.dma_start(out=iota_sb[:], in_=iota.ap())
            dinv_sb = constp.tile([P, nblk], dt.float32)
            nc.sync.dma_start(out=dinv_sb[:], in_=dinv.ap())
            dstrel_sb = constp.tile([P, NT], dt.bfloat16)
            nc.sync.dma_start(out=dstrel_sb[:], in_=dstrel.ap())
            ident = constp.tile([P, P], dt.float32)
            make_identity(nc, ident[:])

            def blk_rows(blk):
                return min(P, nslice - blk * P)

            def rows_of(blks):
                return sum(blk_rows(b) for b in blks)

            def store_rows(dram, base, nb, rl, tile3, width, col0=0):
                """DMA [128, nb, width] tile -> dram rows [base..), cols
                [col0:col0+width), possibly-partial last block (rl rows)."""
                nbf = nb - 1
                if nbf:
                    nc.sync.dma_start(
                        out=dram.ap()[base:base + nbf * P,
                                      col0:col0 + width].rearrange(
                            "(b p) f -> p b f", p=P),
                        in_=tile3[:, :nbf, :])
                pb = base + nbf * P
                nc.sync.dma_start(
                    out=dram.ap()[pb:pb + rl, col0:col0 + width].rearrange(
                        "(b p) f -> p b f", p=rl),
                    in_=tile3[:rl, nbf:nb, :])

            def load_rows(dram, base, nb, rl, tile3, width):
                nbf = nb - 1
                if nbf:
                    nc.sync.dma_start(
                        out=tile3[:, :nbf, :],
                        in_=dram.ap()[base:base + nbf * P, 0:width].rearrange(
                            "(b p) f -> p b f", p=P))
                pb = base + nbf * P
                nc.sync.dma_start(
                    out=tile3[:rl, nbf:nb, :],
                    in_=dram.ap()[pb:pb + rl, 0:width].rearrange(
                        "(b p) f -> p b f", p=rl))

            # ---------- phase B: g0 = dinv * (x @ W1) ----------
            for s_i, blks in enumerate(sch.sgs):
                nb = len(blks)
                rl = blk_rows(blks[-1])
                base = blks[0] * P
                t_sb = epip.tile([P, SG_BLOCKS, FEAT], dt.float32, tag="tsb")
                if rl < P:
                    nc.vector.memset(t_sb[:], 0.0)
                sg_rows = rows_of(blks)
                xT_sb = epip.tile([FEAT, SG_BLOCKS * P], dt.float32, tag="xT")
                nc.sync.dma_start(out=xT_sb[:, :sg_rows],
                                  in_=xT.ap()[:, base:base + sg_rows])
                for j, blk in enumerate(blks):
                    r = blk_rows(blk)
                    ps = psA.tile([P, FEAT], dt.float32, tag="agg")
                    nc.tensor.matmul(ps[:r, :], xT_sb[:, j * P:j * P + r],
                                     W1_sb[:], start=True, stop=True)
                    nc.scalar.mul(t_sb[:r, j, :], ps[:r, :],
                                  dinv_sb[:r, blk:blk + 1])
                # f32 copy for the local self-loop term
                store_rows(g0f, base, nb, rl, t_sb[:, :nb, :], FEAT)
                hi = hilop.tile([P, SG_BLOCKS, FEAT], dt.bfloat16, tag="hi")
                hi32 = hilop.tile([P, SG_BLOCKS, FEAT], dt.float32,
                                  tag="hi32")
                lo = hilop.tile([P, SG_BLOCKS, FEAT], dt.bfloat16, tag="lo")
                nc.vector.tensor_copy(hi[:, :nb, :], t_sb[:, :nb, :])
                nc.vector.tensor_copy(hi32[:, :nb, :], hi[:, :nb, :])
                nc.vector.tensor_tensor(lo[:, :nb, :], t_sb[:, :nb, :],
                                        hi32[:, :nb, :], OP.subtract)
                store_rows(g0_slice, base, nb, rl, hi[:, :nb, :], FEAT, 0)
                store_rows(g0_slice, base, nb, rl, lo[:, :nb, :], FEAT, FEAT)

            tc.no_sync_barrier()
            if phases < 2:
                raise _PhaseDone()
            # ---------- AllGather g0 ----------
            nc.gpsimd.collective_compute(
                "AllGather", OP.bypass, replica_groups=replica_groups,
                ins=[g0_slice.ap().opt()], outs=[g0_full.ap().opt()])

            tc.no_sync_barrier()
            if phases < 3:
                raise _PhaseDone()

            # ---------- aggregation emitter ----------
            def aggregation(layer, g_full):
                for s_i, blks in enumerate(sch.sgs):
                    gts = {}
                    for b_i in range(nbank):
                        L = sch.gather_L[s_i][b_i]
                        if L == 0:
                            continue
                        gt = gatherp.tile([P, maxC, 2 * FEAT], dt.bfloat16,
                                          tag="gt")
                        rows = min(sch.bankrows,
                                   sch.n_nodes - b_i * sch.bankrows)
                        src_ap = g_full.ap()[b_i * sch.bankrows:
                                             b_i * sch.bankrows + rows, :]
                        go = sch.gidx_off[(s_i, b_i)]
                        for q0 in range(0, L, GMAX):
                            q1 = min(L, q0 + GMAX)
                            Lq = q1 - q0
                            it = gidxp.tile([P, GMAX // 16], dt.int16,
                                            tag="gidx")
                            nc.sync.dma_start(
                                out=it[:, :Lq // 16],
                                in_=gidx.ap()[:, go + q0 // 16:
                                              go + q1 // 16])
                            nc.gpsimd.dma_gather(
                                gt[:, q0 // P:q1 // P, :], src_ap,
                                it[:, :Lq // 16], Lq, Lq, 2 * FEAT)
                        gts[b_i] = gt
                    if subph == "gather":
                        continue
                    sbatch, sb_base = None, -100
                    ps = None
                    for (b_i, tc_, dcol, blk, st_, sp_) in sch.mmseq[s_i]:
                        if sbatch is None or dcol - sb_base >= 8:
                            w = min(8, sch.ncols - dcol)
                            sbatch = sp.tile([P, 8, P], dt.bfloat16,
                                             tag="S")
                            sb_base = dcol
                            nc.vector.tensor_tensor(
                                sbatch[:, :w, :],
                                dstrel_sb[:, dcol:dcol + w, None
                                          ].broadcast_to([P, w, P]),
                                iota_sb[:, None, :].broadcast_to([P, w, P]),
                                OP.is_equal)
                        if subph == "sbuild":
                            continue
                        if st_:
                            ps = psA.tile([P, FEAT], dt.float32, tag="agg",
                                          name=f"agg_l{layer}_s{s_i}b{blk}")
                        S_t = sbatch[:, dcol - sb_base, :]
                        rhs_hi = gts[b_i][:, tc_, 0:FEAT]
                        rhs_lo = gts[b_i][:, tc_, FEAT:2 * FEAT]
                        nc.tensor.matmul(ps[:], S_t, rhs_hi,
                                         start=st_, stop=False)
                        nc.tensor.matmul(ps[:], S_t, rhs_lo,
                                         start=False, stop=sp_)
                        if sp_ and subph == "full":
                            yield s_i, blks, blk, ps

            # ---------- layer 1 ----------
            cur_sg, t_sb = -1, None
            for s_i, blks, blk, ps in aggregation(1, g0_full):
                nb = len(blks)
                rl = blk_rows(blks[-1])
                base = blks[0] * P
                if s_i != cur_sg:
                    cur_sg = s_i
                    t_sb = epip.tile([P, SG_BLOCKS, FEAT], dt.float32,
                                     tag="tsb", name=f"l1t_{s_i}")
                    if rl < P:
                        nc.vector.memset(t_sb[:], 0.0)
                j = blks.index(blk)
                r = blk_rows(blk)
                nc.scalar.copy(t_sb[:r, j, :], ps[:r, :])
                if blk == blks[-1]:
                    gfl = epip.tile([P, SG_BLOCKS, FEAT], dt.float32,
                                    tag="gfl", name=f"l1gf_{s_i}")
                    if rl < P:
                        nc.vector.memset(gfl[:], 0.0)
                    load_rows(g0f, base, nb, rl, gfl, FEAT)
                    dv = dinv_sb[:, blks[0]:blks[0] + nb, None].broadcast_to(
                        [P, nb, FEAT])
                    b1b = b1_sb[:, None, :].broadcast_to([P, nb, FEAT])
                    # t = (psum + g0f)*dinv + b1; h1 = relu(t); g1 = h1*dinv
                    nc.vector.tensor_tensor(t_sb[:, :nb, :], t_sb[:, :nb, :],
                                            gfl[:, :nb, :], OP.add)
                    nc.vector.tensor_tensor(t_sb[:, :nb, :], t_sb[:, :nb, :],
                                            dv, OP.mult)
                    nc.vector.tensor_tensor(t_sb[:, :nb, :], t_sb[:, :nb, :],
                                            b1b, OP.add)
                    nc.vector.tensor_scalar_max(t_sb[:, :nb, :],
                                                t_sb[:, :nb, :], 0.0)
                    nc.vector.tensor_tensor(t_sb[:, :nb, :], t_sb[:, :nb, :],
                                            dv, OP.mult)
                    store_rows(g1f, base, nb, rl, t_sb[:, :nb, :], FEAT)
                    hi = hilop.tile([P, SG_BLOCKS, FEAT], dt.bfloat16,
                                    tag="hi")
                    hi32 = hilop.tile([P, SG_BLOCKS, FEAT], dt.float32,
                                      tag="hi32")
                    lo = hilop.tile([P, SG_BLOCKS, FEAT], dt.bfloat16,
                                    tag="lo")
                    nc.vector.tensor_copy(hi[:, :nb, :], t_sb[:, :nb, :])
                    nc.vector.tensor_copy(hi32[:, :nb, :], hi[:, :nb, :])
                    nc.vector.tensor_tensor(lo[:, :nb, :], t_sb[:, :nb, :],
                                            hi32[:, :nb, :], OP.subtract)
                    store_rows(g1_slice, base, nb, rl, hi[:, :nb, :],
                               FEAT, 0)
                    store_rows(g1_slice, base, nb, rl, lo[:, :nb, :],
                               FEAT, FEAT)

            tc.no_sync_barrier()
            if phases < 4:
                raise _PhaseDone()
            # ---------- AllGather g1 ----------
            nc.gpsimd.collective_compute(
                "AllGather", OP.bypass, replica_groups=replica_groups,
                ins=[g1_slice.ap().opt()], outs=[g1_full.ap().opt()])

            tc.no_sync_barrier()
            if phases < 5:
                raise _PhaseDone()

            # ---------- layer 2 + head ----------
            cur_sg, z_sb = -1, None
            for s_i, blks, blk, ps in aggregation(2, g1_full):
                nb = len(blks)
                rl = blk_rows(blks[-1])
                base = blks[0] * P
                if s_i != cur_sg:
                    cur_sg = s_i
                    z_sb = epip.tile([P, SG_BLOCKS, OUTC], dt.float32,
                                     tag="zsb", name=f"z_{s_i}")
                    if rl < P:
                        nc.vector.memset(z_sb[:], 0.0)
                j = blks.index(blk)
                r = blk_rows(blk)
                gfb = epip.tile([P, FEAT], dt.float32, tag="gfb")
                nc.sync.dma_start(out=gfb[:r, :],
                                  in_=g1f.ap()[blk * P:blk * P + r, :])
                traw = epip.tile([P, FEAT], dt.float32, tag="traw")
                nc.vector.tensor_tensor(traw[:r, :], ps[:r, :], gfb[:r, :],
                                        OP.add)
                pst = psT.tile([FEAT, P], dt.float32, tag="pst")
                nc.tensor.transpose(pst[:, :r], traw[:r, :], ident[:r, :r])
                tT = epip.tile([FEAT, P], dt.float32, tag="tT")
                nc.scalar.copy(tT[:, :r], pst[:, :r])
                psz = psZ.tile([P, OUTC], dt.float32, tag="psz")
                nc.tensor.matmul(psz[:r, :], tT[:, :r], W2_sb[:],
                                 start=True, stop=True)
                nc.scalar.mul(z_sb[:r, j, :], psz[:r, :],
                              dinv_sb[:r, blk:blk + 1])
                if blk == blks[-1]:
                    b2b = b2_sb[:, None, :].broadcast_to([P, nb, OUTC])
                    nc.vector.tensor_tensor(z_sb[:, :nb, :], z_sb[:, :nb, :],
                                            b2b, OP.add)
                    mx = epip.tile([P, SG_BLOCKS], dt.float32, tag="mx")
                    nc.vector.tensor_reduce(
                        mx[:, :nb], z_sb[:, :nb, :],
                        axis=mybir.AxisListType.X, op=OP.max)
                    mxb = mx[:, :nb, None].broadcast_to([P, nb, OUTC])
                    nc.vector.tensor_tensor(z_sb[:, :nb, :], z_sb[:, :nb, :],
                                            mxb, OP.subtract)
                    ex = epip.tile([P, SG_BLOCKS, OUTC], dt.float32, tag="ex")
                    nc.scalar.activation(ex[:, :nb, :], z_sb[:, :nb, :],
                                         AF.Exp)
                    sm = epip.tile([P, SG_BLOCKS], dt.float32, tag="sm")
                    nc.vector.tensor_reduce(
                        sm[:, :nb], ex[:, :nb, :],
                        axis=mybir.AxisListType.X, op=OP.add)
                    lse = epip.tile([P, SG_BLOCKS], dt.float32, tag="lse")
                    nc.scalar.activation(lse[:, :nb], sm[:, :nb], AF.Ln)
                    lseb = lse[:, :nb, None].broadcast_to([P, nb, OUTC])
                    nc.vector.tensor_tensor(z_sb[:, :nb, :], z_sb[:, :nb, :],
                                            lseb, OP.subtract)
                    store_rows(z_out, base, nb, rl, z_sb[:, :nb, :], OUTC)
          except _PhaseDone:
            pass

    nc.compile()
    return nc


# --------------------------------------------------------------------------
# Entry point
# --------------------------------------------------------------------------

_cache = {}


def make_in_maps(sch, x, dst, W1, b1, W2, b2):
    n_nodes = sch.n_nodes
    deg = np.bincount(dst, minlength=n_nodes).astype(np.float32) + 1.0
    dinv = 1.0 / np.sqrt(deg)
    nslice, nblk = sch.nslice, sch.nblk
    in_maps = []
    iota_np = np.tile(np.arange(P, dtype=np.float32)[None, :], (P, 1))
    for c in range(sch.n_cores):
        xs = x[c * nslice:(c + 1) * nslice].astype(np.float32)
        dv = dinv[c * nslice:(c + 1) * nslice]
        dv_pad = np.ones(nblk * P, np.float32)
        dv_pad[:nslice] = dv
        gi = sch.core_gidx[c]
        if gi.shape[1] < 16:
            gi = np.zeros((P, 16), np.int16)
        in_maps.append({
            "xT": np.ascontiguousarray(xs.T),
            "W1": np.asarray(W1, np.float32),
            "W2": np.asarray(W2, np.float32),
            "b1r": np.tile(np.asarray(b1, np.float32)[None, :], (P, 1)),
            "b2r": np.tile(np.asarray(b2, np.float32)[None, :], (P, 1)),
            "iota": _to_bf16(iota_np),
            "dinv": np.ascontiguousarray(dv_pad.reshape(nblk, P).T),
            "gidx": gi,
            "dstrel": _to_bf16(sch.core_dstrel[c]),
        })
    return in_maps


def gcn_reference_np(x, src, dst, W1, b1, W2, b2):
    n = x.shape[0]
    deg = np.bincount(dst, minlength=n).astype(np.float32) + 1.0
    dinv = 1.0 / np.sqrt(deg)

    def conv(h, W, b):
        h = h @ W
        norm = dinv[src] * dinv[dst]
        agg = np.zeros_like(h)
        np.add.at(agg, dst, h[src] * norm[:, None])
        agg = agg + h * (dinv * dinv)[:, None]
        return agg + b

    h = np.maximum(conv(x, W1, b1), 0.0)
    z = conv(h, W2, b2)
    z = z - z.max(axis=1, keepdims=True)
    return z - np.log(np.exp(z).sum(axis=1, keepdims=True))


def _ensure_ntff_hook():
    import types
    try:
        from antenv import axon_hooks  # noqa: F401
        return
    except ImportError:
        pass
    try:
        from trn_agent_boot.trn_boot import _ntff_profile_via_ctypes
        hook = _ntff_profile_via_ctypes("/opt/axon/libaxon_pjrt.so")
        m = types.ModuleType("antenv.axon_hooks")
        m.get_axon_ntff_profile_hook = lambda: hook
        m.set_axon_ntff_profile_hook = lambda h: None
        sys.modules["antenv.axon_hooks"] = m
    except Exception:
        pass


def _to_bf16(a):
    import ml_dtypes
    return a.astype(ml_dtypes.bfloat16)


def kernel(x, edge_index, W1, b1, W2, b2):
    _phases = int(os.environ.get("GCN_PHASES", "5"))
    x = np.asarray(x)
    edge_index = np.asarray(edge_index)
    n_nodes = x.shape[0]
    n_cores = 8
    src = edge_index[0].astype(np.int64)
    dst = edge_index[1].astype(np.int64)

    ck = (n_nodes, edge_index.shape[1],
          int(edge_index[:, :100].sum()), int(edge_index[:, -100:].sum()))
    if ck in _cache:
        sch, nc = _cache[ck]
    else:
        sch = build_schedule(src, dst, n_nodes, n_cores)
        nc = build_program(sch, phases=_phases)
        _cache[ck] = (sch, nc)

    in_maps = make_in_maps(sch, x, dst, W1, b1, W2, b2)

    from concourse.bass_utils import run_bass_kernel_spmd
    trace = bool(int(os.environ.get("GCN_TRACE", "0")))
    if trace:
        _ensure_ntff_hook()
    try:
        res = run_bass_kernel_spmd(nc, in_maps, core_ids=list(range(n_cores)),
                                   trace=trace)
    except Exception:
        if not trace:
            raise
        res = run_bass_kernel_spmd(nc, in_maps, core_ids=list(range(n_cores)),
                                   trace=False)
    kernel._last_results = res
    out = np.concatenate([res.results[c]["z"] for c in range(n_cores)],
                         axis=0)
    return out.astype(np.float32)


if __name__ == "__main__":
    rng = np.random.default_rng(0)
    N, E = 4096, 60000
    src = rng.integers(0, N, E)
    dst = rng.integers(0, N, E)
    sch = build_schedule(src, dst, N, 8)
    print("ncols", sch.ncols, "gidx_cols", sch.gidx_cols, "maxL", sch.maxL)
    print("schedule numpy check rel err:",
          numpy_check_schedule(sch, src, dst, N))



# revision 3
# speedup vs baseline: 2.3866x; 2.3866x over previous
"""Trainium2 Bass kernel for a 2-layer GCN (GCNConv -> ReLU -> GCNConv -> log_softmax).

Strategy (8 NeuronCores, SPMD, one NEFF):
  * Nodes are range-sharded across the 8 cores by destination. Each core owns
    N/8 destination nodes and all edges pointing at them.
  * GCN symmetric normalization factorizes: out = D^-1/2 (A+I) D^-1/2 h.
    Rows are pre-scaled by dinv once per node, aggregated unnormalized, and
    post-scaled by dinv. The self-loop term is added locally from an f32
    copy of the core's own rows (never gathered).
  * The gather h[src] uses dma_gather of 256-byte rows stored as
    [bf16(hi) | bf16(lo)] pairs; hi/lo splitting keeps ~fp32 precision while
    the segment-sum matmuls run at bf16 speed.
  * dma_gather descriptor generation is Q7-serial (~7.5 ns/idx) and a single
    instruction must stay <= 1024 indices (SWDGE ring limit), so gathers are
    emitted as sub-gathers of <= 1024 idxs writing disjoint slices of one
    SBUF tile.
  * Segment-sum: edges are grouped per (dst 128-node block, src bank); a
    one-hot S matrix [128 edges x 128 node slots] is built on the Vector
    engine (is_equal vs an iota row, batched) and the TensorEngine computes
    S^T @ msg accumulating into a PSUM tile per dst block.
  * dma_gather indices are int16 -> the gather source is split into 4 banks
    of 25000 rows.
  * One NEFF runs on all 8 cores: tiles-per-(block,bank) is fixed to the max
    over the 8 cores (pad slots use idx=0 and dst_rel=-1 -> zero S rows).
  * Between layers the per-core h1 slices are AllGathered (HBM collective).
"""

import os
import sys
import numpy as np

P = 128          # partitions / edge-tile size
FEAT = 64        # hidden feature width
OUTC = 32        # output classes
SG_BLOCKS = 6    # dst blocks per supergroup (psum pool is 6 + 2 spare banks)
MAX_BANK_ROWS = 25000   # int16 gather index limit
GMAX = 1024      # max idxs per dma_gather instruction (SWDGE ring limit)


class _PhaseDone(Exception):
    pass


class Schedule:
    pass


# --------------------------------------------------------------------------
# Host-side schedule construction
# --------------------------------------------------------------------------

def build_schedule(src, dst, n_nodes, n_cores):
    """64-slot-quantized runs per (block, bank); 128-edge tiles may span two
    adjacent runs (two matmul targets). Schedule is shared by all cores
    (SPMD): run lengths = 64*ceil(max-over-cores/64)."""
    assert n_nodes % n_cores == 0
    Q = 64
    nslice = n_nodes // n_cores
    nblk = (nslice + P - 1) // P
    nbank = (n_nodes + MAX_BANK_ROWS - 1) // MAX_BANK_ROWS
    bankrows = (n_nodes + nbank - 1) // nbank
    assert bankrows <= 32767

    src_a = src.astype(np.int64)
    dst_a = dst.astype(np.int64)
    core = dst_a // nslice
    block = (dst_a % nslice) // P
    bank = src_a // bankrows

    key = (core * nblk + block) * nbank + bank
    counts = np.bincount(key, minlength=n_cores * nblk * nbank).reshape(
        n_cores, nblk, nbank)
    R = (np.ceil(counts.max(axis=0) / Q) * Q).astype(np.int64)  # slots/cell
    for b in range(nblk):
        if R[b].sum() == 0:
            R[b, 0] = Q

    sgs = [list(range(sgi, min(sgi + SG_BLOCKS, nblk)))
           for sgi in range(0, nblk, SG_BLOCKS)]

    sch = Schedule()
    sch.n_nodes, sch.n_cores, sch.nslice = n_nodes, n_cores, nslice
    sch.nblk, sch.nbank, sch.bankrows = nblk, nbank, bankrows
    sch.sgs = sgs

    # per (sg, bank): run slot offsets, LR (total slots, padded to 128)
    sch.run_off = []      # [s_i][(blk,b)] -> slot offset within (sg,bank)
    sch.gather_L = []
    for s_i, blks in enumerate(sgs):
        offs = {}
        Ls = []
        for b_i in range(nbank):
            o = 0
            for blk in blks:
                offs[(blk, b_i)] = o
                o += int(R[blk, b_i])
            o = ((o + P - 1) // P) * P   # pad gather to 128-multiple
            Ls.append(o)
        sch.run_off.append(offs)
        sch.gather_L.append(Ls)
    sch.maxL = max(max(Ls) for Ls in sch.gather_L) if sgs else 0

    # matmul sequence per sg: block-major; for each block, for each bank,
    # the tiles overlapping its run (boundary tiles shared with neighbours).
    # Each entry: (bank, tile_col, dcol, blk, start, stop)
    sch.mmseq = []        # [s_i] -> list of entries
    ncols = 0
    dcol_map = []         # (s_i, bank, tile_col, blk) per dstrel column
    for s_i, blks in enumerate(sgs):
        seq = []
        per_block = {blk: [] for blk in blks}
        for blk in blks:
            for b_i in range(nbank):
                r0 = sch.run_off[s_i][(blk, b_i)]
                r1 = r0 + int(R[blk, b_i])
                if r1 == r0:
                    continue
                t0, t1 = r0 // P, (r1 - 1) // P
                for tc_ in range(t0, t1 + 1):
                    per_block[blk].append((b_i, tc_))
        for blk in blks:
            lst = per_block[blk]
            for i, (b_i, tc_) in enumerate(lst):
                seq.append([b_i, tc_, ncols, blk, i == 0, i == len(lst) - 1])
                dcol_map.append((s_i, b_i, tc_, blk))
                ncols += 1
        sch.mmseq.append(seq)
    sch.ncols = ncols
    sch.R = R

    # gidx layout
    off = 0
    gidx_off = {}
    for s_i in range(len(sgs)):
        for b_i in range(nbank):
            gidx_off[(s_i, b_i)] = off
            off += sch.gather_L[s_i][b_i] // 16
    sch.gidx_cols = off
    sch.gidx_off = gidx_off

    # ---------------- per-core arrays ----------------
    order = np.lexsort((bank, block, core))
    s_o, d_o = src_a[order], dst_a[order]
    grp_key = (core[order] * nblk + block[order]) * nbank + bank[order]
    uniq, starts = np.unique(grp_key, return_index=True)
    grp_start = {int(k): int(v) for k, v in zip(uniq, starts)}
    grp_count = {int(k): int(v) for k, v in
                 zip(uniq, np.diff(np.append(starts, len(grp_key))))}

    sch.core_gidx = []
    sch.core_dstrel = []
    for c in range(n_cores):
        gidx = np.zeros((16, sch.gidx_cols), dtype=np.int16)
        dstrel = np.full((P, sch.ncols), -1.0, dtype=np.float32)
        # edge slot data per (sg, bank)
        slot_src = {}
        slot_dst = {}
        for s_i, blks in enumerate(sgs):
            for b_i in range(nbank):
                L = sch.gather_L[s_i][b_i]
                if L == 0:
                    continue
                idx_lin = np.zeros(L, dtype=np.int16)
                dst_lin = np.full(L, -1, dtype=np.int64)
                for blk in blks:
                    k = int((c * nblk + blk) * nbank + b_i)
                    cnt = grp_count.get(k, 0)
                    if not cnt:
                        continue
                    st = grp_start.get(k, 0)
                    o = sch.run_off[s_i][(blk, b_i)]
                    idx_lin[o:o + cnt] = (
                        s_o[st:st + cnt] - b_i * bankrows).astype(np.int16)
                    dst_lin[o:o + cnt] = d_o[st:st + cnt]
                go = gidx_off[(s_i, b_i)]
                gidx[:, go:go + L // 16] = idx_lin.reshape(L // 16, 16).T
                slot_src[(s_i, b_i)] = idx_lin
                slot_dst[(s_i, b_i)] = dst_lin
        # dstrel per (tile, target-block) column
        for dcol, (s_i, b_i, tc_, blk) in enumerate(dcol_map):
            dl = slot_dst.get((s_i, b_i))
            if dl is None:
                continue
            seg = dl[tc_ * P:(tc_ + 1) * P]
            base = c * nslice + blk * P
            vals = seg - base
            vals = np.where((seg >= 0) & (vals >= 0) & (vals < P),
                            vals, -1).astype(np.float32)
            dstrel[:len(seg), dcol] = vals
        sch.core_gidx.append(np.tile(gidx, (8, 1)))
        sch.core_dstrel.append(dstrel)
    return sch


def numpy_check_schedule(sch, src, dst, n_nodes):
    """Emulate the device aggregation (no self loops) in numpy."""
    rng = np.random.default_rng(0)
    g = rng.standard_normal((n_nodes, FEAT)).astype(np.float32)
    ref = np.zeros((n_nodes, FEAT), np.float32)
    np.add.at(ref, dst, g[src])
    out = np.zeros((n_nodes, FEAT), np.float32)
    for c in range(sch.n_cores):
        gidx = sch.core_gidx[c]
        dstrel = sch.core_dstrel[c]
        msgs = {}
        for s_i in range(len(sch.sgs)):
            for b_i in range(sch.nbank):
                L = sch.gather_L[s_i][b_i]
                if L == 0:
                    continue
                go = sch.gidx_off[(s_i, b_i)]
                idx = gidx[:16, go:go + L // 16].T.reshape(-1)
                rows = g[b_i * sch.bankrows:
                         min((b_i + 1) * sch.bankrows, n_nodes)]
                msgs[(s_i, b_i)] = rows[idx]
        for s_i in range(len(sch.sgs)):
            for (b_i, tc_, dcol, blk, start, stop) in sch.mmseq[s_i]:
                m = msgs[(s_i, b_i)][tc_ * P:(tc_ + 1) * P]
                S = (dstrel[:, dcol][:, None] ==
                     np.arange(P)[None, :]).astype(np.float32)
                base = c * sch.nslice + blk * P
                hi = min(base + P, n_nodes)
                out[base:hi] += (S.T @ m)[:hi - base]
    return np.abs(out - ref).max() / (np.abs(ref).max() + 1e-9)


# --------------------------------------------------------------------------
# Bass program
# --------------------------------------------------------------------------

def build_program(sch, phases=5):
    import concourse.mybir as mybir
    import concourse.tile as tile
    from concourse import bacc
    from concourse.masks import make_identity

    dt = mybir.dt
    AF = mybir.ActivationFunctionType
    OP = mybir.AluOpType

    n_cores = sch.n_cores
    nslice, nblk, nbank = sch.nslice, sch.nblk, sch.nbank
    NT = sch.ncols
    subph = os.environ.get("GCN_SUBPH", "full")

    nc = bacc.Bacc("TRN2", target_bir_lowering=False, debug=False,
                   num_devices=n_cores, num_swdge_queues=4)

    xT = nc.dram_tensor("xT", [FEAT, nslice], dt.float32, kind="ExternalInput")
    W1 = nc.dram_tensor("W1", [FEAT, FEAT], dt.float32, kind="ExternalInput")
    W2 = nc.dram_tensor("W2", [FEAT, OUTC], dt.float32, kind="ExternalInput")
    b1r = nc.dram_tensor("b1r", [P, FEAT], dt.float32, kind="ExternalInput")
    b2r = nc.dram_tensor("b2r", [P, OUTC], dt.float32, kind="ExternalInput")
    iota = nc.dram_tensor("iota", [P, P], dt.bfloat16, kind="ExternalInput")
    dinv = nc.dram_tensor("dinv", [P, nblk], dt.float32, kind="ExternalInput")
    gidx = nc.dram_tensor("gidx", [P, max(sch.gidx_cols, 16)], dt.int16,
                          kind="ExternalInput")
    dstrel = nc.dram_tensor("dstrel", [P, NT], dt.bfloat16,
                            kind="ExternalInput")
    z_out = nc.dram_tensor("z", [nslice, OUTC], dt.float32,
                           kind="ExternalOutput")

    g0_slice = nc.dram_tensor("g0_slice", [nslice, 2 * FEAT], dt.bfloat16)
    g0f = nc.dram_tensor("g0f", [nslice, FEAT], dt.float32)
    g1f = nc.dram_tensor("g1f", [nslice, FEAT], dt.float32)
    g0_full = nc.dram_tensor("g0_full", [sch.n_nodes, 2 * FEAT], dt.bfloat16,
                             addr_space="Shared")
    g1_slice = nc.dram_tensor("g1_slice", [nslice, 2 * FEAT], dt.bfloat16)
    g1_full = nc.dram_tensor("g1_full", [sch.n_nodes, 2 * FEAT], dt.bfloat16,
                             addr_space="Shared")

    replica_groups = [list(range(n_cores))]
    maxC = sch.maxL // P

    with tile.TileContext(nc) as tc:
        with (
            tc.tile_pool(name="const", bufs=1) as constp,
            tc.tile_pool(name="gather", bufs=nbank + 2) as gatherp,
            tc.tile_pool(name="sbuild", bufs=4) as sp,
            tc.tile_pool(name="gidxp", bufs=2 * (nbank + 2)) as gidxp,
            tc.tile_pool(name="epi", bufs=3) as epip,
            tc.tile_pool(name="hilo", bufs=3) as hilop,
            tc.tile_pool(name="psA", bufs=6, space="PSUM") as psA,
            tc.tile_pool(name="psT", bufs=1, space="PSUM") as psT,
            tc.tile_pool(name="psZ", bufs=1, space="PSUM") as psZ,
        ):
          try:
            W1_sb = constp.tile([FEAT, FEAT], dt.float32)
            nc.sync.dma_start(out=W1_sb[:], in_=W1.ap())
            W2_sb = constp.tile([FEAT, OUTC], dt.float32)
            nc.sync.dma_start(out=W2_sb[:], in_=W2.ap())
            b1_sb = constp.tile([P, FEAT], dt.float32)
            nc.sync.dma_start(out=b1_sb[:], in_=b1r.ap())
            b2_sb = constp.tile([P, OUTC], dt.float32)
            nc.sync.dma_start(out=b2_sb[:], in_=b2r.ap())
            iota_sb = constp.tile([P, P], dt.bfloat16)
            nc.sync.dma_start(out=iota_sb[:], in_=iota.ap())
            dinv_sb = constp.tile([P, nblk], dt.float32)
            nc.sync.dma_start(out=dinv_sb[:], in_=dinv.ap())
            dstrel_sb = constp.tile([P, NT], dt.bfloat16)
            nc.sync.dma_start(out=dstrel_sb[:], in_=dstrel.ap())
            ident = constp.tile([P, P], dt.float32)
            make_identity(nc, ident[:])

            def blk_rows(blk):
                return min(P, nslice - blk * P)

            def rows_of(blks):
                return sum(blk_rows(b) for b in blks)

            def store_rows(dram, base, nb, rl, tile3, width, col0=0):
                """DMA [128, nb, width] tile -> dram rows [base..), cols
                [col0:col0+width), possibly-partial last block (rl rows)."""
                nbf = nb - 1
                if nbf:
                    nc.sync.dma_start(
                        out=dram.ap()[base:base + nbf * P,
                                      col0:col0 + width].rearrange(
                            "(b p) f -> p b f", p=P),
                        in_=tile3[:, :nbf, :])
                pb = base + nbf * P
                nc.sync.dma_start(
                    out=dram.ap()[pb:pb + rl, col0:col0 + width].rearrange(
                        "(b p) f -> p b f", p=rl),
                    in_=tile3[:rl, nbf:nb, :])

            def load_rows(dram, base, nb, rl, tile3, width):
                nbf = nb - 1
                if nbf:
                    nc.sync.dma_start(
                        out=tile3[:, :nbf, :],
                        in_=dram.ap()[base:base + nbf * P, 0:width].rearrange(
                            "(b p) f -> p b f", p=P))
                pb = base + nbf * P
                nc.sync.dma_start(
                    out=tile3[:rl, nbf:nb, :],
                    in_=dram.ap()[pb:pb + rl, 0:width].rearrange(
                        "(b p) f -> p b f", p=rl))

            # ---------- phase B: g0 = dinv * (x @ W1) ----------
            for s_i, blks in enumerate(sch.sgs):
                nb = len(blks)
                rl = blk_rows(blks[-1])
                base = blks[0] * P
                t_sb = epip.tile([P, SG_BLOCKS, FEAT], dt.float32, tag="tsb")
                if rl < P:
                    nc.vector.memset(t_sb[:], 0.0)
                sg_rows = rows_of(blks)
                xT_sb = epip.tile([FEAT, SG_BLOCKS * P], dt.float32, tag="xT")
                nc.sync.dma_start(out=xT_sb[:, :sg_rows],
                                  in_=xT.ap()[:, base:base + sg_rows])
                for j, blk in enumerate(blks):
                    r = blk_rows(blk)
                    ps = psA.tile([P, FEAT], dt.float32, tag="agg")
                    nc.tensor.matmul(ps[:r, :], xT_sb[:, j * P:j * P + r],
                                     W1_sb[:], start=True, stop=True)
                    nc.scalar.mul(t_sb[:r, j, :], ps[:r, :],
                                  dinv_sb[:r, blk:blk + 1])
                # f32 copy for the local self-loop term
                store_rows(g0f, base, nb, rl, t_sb[:, :nb, :], FEAT)
                hi = hilop.tile([P, SG_BLOCKS, FEAT], dt.bfloat16, tag="hi")
                hi32 = hilop.tile([P, SG_BLOCKS, FEAT], dt.float32,
                                  tag="hi32")
                lo = hilop.tile([P, SG_BLOCKS, FEAT], dt.bfloat16, tag="lo")
                nc.vector.tensor_copy(hi[:, :nb, :], t_sb[:, :nb, :])
                nc.vector.tensor_copy(hi32[:, :nb, :], hi[:, :nb, :])
                nc.vector.tensor_tensor(lo[:, :nb, :], t_sb[:, :nb, :],
                                        hi32[:, :nb, :], OP.subtract)
                store_rows(g0_slice, base, nb, rl, hi[:, :nb, :], FEAT, 0)
                store_rows(g0_slice, base, nb, rl, lo[:, :nb, :], FEAT, FEAT)

            tc.no_sync_barrier()
            if phases < 2:
                raise _PhaseDone()
            # ---------- AllGather g0 ----------
            nc.gpsimd.collective_compute(
                "AllGather", OP.bypass, replica_groups=replica_groups,
                ins=[g0_slice.ap().opt()], outs=[g0_full.ap().opt()])

            tc.no_sync_barrier()
            if phases < 3:
                raise _PhaseDone()

            # ---------- aggregation emitter ----------
            qn_counter = [0]

            def aggregation(layer, g_full):
                for s_i, blks in enumerate(sch.sgs):
                    gts = {}
                    for b_i in range(nbank):
                        L = sch.gather_L[s_i][b_i]
                        if L == 0:
                            continue
                        gt = gatherp.tile([P, maxC, 2 * FEAT], dt.bfloat16,
                                          tag="gt")
                        rows = min(sch.bankrows,
                                   sch.n_nodes - b_i * sch.bankrows)
                        src_ap = g_full.ap()[b_i * sch.bankrows:
                                             b_i * sch.bankrows + rows, :]
                        go = sch.gidx_off[(s_i, b_i)]
                        for q0 in range(0, L, GMAX):
                            q1 = min(L, q0 + GMAX)
                            Lq = q1 - q0
                            it = gidxp.tile([P, GMAX // 16], dt.int16,
                                            tag="gidx")
                            nc.sync.dma_start(
                                out=it[:, :Lq // 16],
                                in_=gidx.ap()[:, go + q0 // 16:
                                              go + q1 // 16])
                            nc.gpsimd.dma_gather(
                                gt[:, q0 // P:q1 // P, :], src_ap,
                                it[:, :Lq // 16], Lq, Lq, 2 * FEAT,
                                queue_num=qn_counter[0] % 4)
                            qn_counter[0] += 1
                        gts[b_i] = gt
                    if subph == "gather":
                        continue
                    sbatch, sb_base = None, -100
                    ps = None
                    for (b_i, tc_, dcol, blk, st_, sp_) in sch.mmseq[s_i]:
                        if sbatch is None or dcol - sb_base >= 8:
                            w = min(8, sch.ncols - dcol)
                            sbatch = sp.tile([P, 8, P], dt.bfloat16,
                                             tag="S")
                            sb_base = dcol
                            nc.vector.tensor_tensor(
                                sbatch[:, :w, :],
                                dstrel_sb[:, dcol:dcol + w, None
                                          ].broadcast_to([P, w, P]),
                                iota_sb[:, None, :].broadcast_to([P, w, P]),
                                OP.is_equal)
                        if subph == "sbuild":
                            continue
                        if st_:
                            ps = psA.tile([P, FEAT], dt.float32, tag="agg",
                                          name=f"agg_l{layer}_s{s_i}b{blk}")
                        S_t = sbatch[:, dcol - sb_base, :]
                        rhs_hi = gts[b_i][:, tc_, 0:FEAT]
                        rhs_lo = gts[b_i][:, tc_, FEAT:2 * FEAT]
                        nc.tensor.matmul(ps[:], S_t, rhs_hi,
                                         start=st_, stop=False)
                        nc.tensor.matmul(ps[:], S_t, rhs_lo,
                                         start=False, stop=sp_)
                        if sp_ and subph == "full":
                            yield s_i, blks, blk, ps

            # ---------- layer 1 ----------
            cur_sg, t_sb = -1, None
            for s_i, blks, blk, ps in aggregation(1, g0_full):
                nb = len(blks)
                rl = blk_rows(blks[-1])
                base = blks[0] * P
                if s_i != cur_sg:
                    cur_sg = s_i
                    t_sb = epip.tile([P, SG_BLOCKS, FEAT], dt.float32,
                                     tag="tsb", name=f"l1t_{s_i}")
                    if rl < P:
                        nc.vector.memset(t_sb[:], 0.0)
                j = blks.index(blk)
                r = blk_rows(blk)
                nc.scalar.copy(t_sb[:r, j, :], ps[:r, :])
                if blk == blks[-1]:
                    gfl = epip.tile([P, SG_BLOCKS, FEAT], dt.float32,
                                    tag="gfl", name=f"l1gf_{s_i}")
                    if rl < P:
                        nc.vector.memset(gfl[:], 0.0)
                    load_rows(g0f, base, nb, rl, gfl, FEAT)
                    dv = dinv_sb[:, blks[0]:blks[0] + nb, None].broadcast_to(
                        [P, nb, FEAT])
                    b1b = b1_sb[:, None, :].broadcast_to([P, nb, FEAT])
                    # t = (psum + g0f)*dinv + b1; h1 = relu(t); g1 = h1*dinv
                    nc.vector.tensor_tensor(t_sb[:, :nb, :], t_sb[:, :nb, :],
                                            gfl[:, :nb, :], OP.add)
                    nc.vector.tensor_tensor(t_sb[:, :nb, :], t_sb[:, :nb, :],
                                            dv, OP.mult)
                    nc.vector.tensor_tensor(t_sb[:, :nb, :], t_sb[:, :nb, :],
                                            b1b, OP.add)
                    nc.vector.tensor_scalar_max(t_sb[:, :nb, :],
                                                t_sb[:, :nb, :], 0.0)
                    nc.vector.tensor_tensor(t_sb[:, :nb, :], t_sb[:, :nb, :],
                                            dv, OP.mult)
                    store_rows(g1f, base, nb, rl, t_sb[:, :nb, :], FEAT)
                    hi = hilop.tile([P, SG_BLOCKS, FEAT], dt.bfloat16,
                                    tag="hi")
                    hi32 = hilop.tile([P, SG_BLOCKS, FEAT], dt.float32,
                                      tag="hi32")
                    lo = hilop.tile([P, SG_BLOCKS, FEAT], dt.bfloat16,
                                    tag="lo")
                    nc.vector.tensor_copy(hi[:, :nb, :], t_sb[:, :nb, :])
                    nc.vector.tensor_copy(hi32[:, :nb, :], hi[:, :nb, :])
                    nc.vector.tensor_tensor(lo[:, :nb, :], t_sb[:, :nb, :],
                                            hi32[:, :nb, :], OP.subtract)
                    store_rows(g1_slice, base, nb, rl, hi[:, :nb, :],
                               FEAT, 0)
                    store_rows(g1_slice, base, nb, rl, lo[:, :nb, :],
                               FEAT, FEAT)

            tc.no_sync_barrier()
            if phases < 4:
                raise _PhaseDone()
            # ---------- AllGather g1 ----------
            nc.gpsimd.collective_compute(
                "AllGather", OP.bypass, replica_groups=replica_groups,
                ins=[g1_slice.ap().opt()], outs=[g1_full.ap().opt()])

            tc.no_sync_barrier()
            if phases < 5:
                raise _PhaseDone()

            # ---------- layer 2 + head ----------
            cur_sg, z_sb = -1, None
            for s_i, blks, blk, ps in aggregation(2, g1_full):
                nb = len(blks)
                rl = blk_rows(blks[-1])
                base = blks[0] * P
                if s_i != cur_sg:
                    cur_sg = s_i
                    z_sb = epip.tile([P, SG_BLOCKS, OUTC], dt.float32,
                                     tag="zsb", name=f"z_{s_i}")
                    if rl < P:
                        nc.vector.memset(z_sb[:], 0.0)
                j = blks.index(blk)
                r = blk_rows(blk)
                gfb = epip.tile([P, FEAT], dt.float32, tag="gfb")
                nc.sync.dma_start(out=gfb[:r, :],
                                  in_=g1f.ap()[blk * P:blk * P + r, :])
                traw = epip.tile([P, FEAT], dt.float32, tag="traw")
                nc.vector.tensor_tensor(traw[:r, :], ps[:r, :], gfb[:r, :],
                                        OP.add)
                pst = psT.tile([FEAT, P], dt.float32, tag="pst")
                nc.tensor.transpose(pst[:, :r], traw[:r, :], ident[:r, :r])
                tT = epip.tile([FEAT, P], dt.float32, tag="tT")
                nc.scalar.copy(tT[:, :r], pst[:, :r])
                psz = psZ.tile([P, OUTC], dt.float32, tag="psz")
                nc.tensor.matmul(psz[:r, :], tT[:, :r], W2_sb[:],
                                 start=True, stop=True)
                nc.scalar.mul(z_sb[:r, j, :], psz[:r, :],
                              dinv_sb[:r, blk:blk + 1])
                if blk == blks[-1]:
                    b2b = b2_sb[:, None, :].broadcast_to([P, nb, OUTC])
                    nc.vector.tensor_tensor(z_sb[:, :nb, :], z_sb[:, :nb, :],
                                            b2b, OP.add)
                    mx = epip.tile([P, SG_BLOCKS], dt.float32, tag="mx")
                    nc.vector.tensor_reduce(
                        mx[:, :nb], z_sb[:, :nb, :],
                        axis=mybir.AxisListType.X, op=OP.max)
                    mxb = mx[:, :nb, None].broadcast_to([P, nb, OUTC])
                    nc.vector.tensor_tensor(z_sb[:, :nb, :], z_sb[:, :nb, :],
                                            mxb, OP.subtract)
                    ex = epip.tile([P, SG_BLOCKS, OUTC], dt.float32, tag="ex")
                    nc.scalar.activation(ex[:, :nb, :], z_sb[:, :nb, :],
                                         AF.Exp)
                    sm = epip.tile([P, SG_BLOCKS], dt.float32, tag="sm")
                    nc.vector.tensor_reduce(
                        sm[:, :nb], ex[:, :nb, :],
                        axis=mybir.AxisListType.X, op=OP.add)
                    lse = epip.tile([P, SG_BLOCKS], dt.float32, tag="lse")
                    nc.scalar.activation(lse[:, :nb], sm[:, :nb], AF.Ln)
                    lseb = lse[:, :nb, None].broadcast_to([P, nb, OUTC])
                    nc.vector.tensor_tensor(z_sb[:, :nb, :], z_sb[:, :nb, :],
                                            lseb, OP.subtract)
                    store_rows(z_out, base, nb, rl, z_sb[:, :nb, :], OUTC)
          except _PhaseDone:
            pass

    nc.compile()
    return nc


# --------------------------------------------------------------------------
# Entry point
# --------------------------------------------------------------------------

_cache = {}


def make_in_maps(sch, x, dst, W1, b1, W2, b2):
    n_nodes = sch.n_nodes
    deg = np.bincount(dst, minlength=n_nodes).astype(np.float32) + 1.0
    dinv = 1.0 / np.sqrt(deg)
    nslice, nblk = sch.nslice, sch.nblk
    in_maps = []
    iota_np = np.tile(np.arange(P, dtype=np.float32)[None, :], (P, 1))
    for c in range(sch.n_cores):
        xs = x[c * nslice:(c + 1) * nslice].astype(np.float32)
        dv = dinv[c * nslice:(c + 1) * nslice]
        dv_pad = np.ones(nblk * P, np.float32)
        dv_pad[:nslice] = dv
        gi = sch.core_gidx[c]
        if gi.shape[1] < 16:
            gi = np.zeros((P, 16), np.int16)
        in_maps.append({
            "xT": np.ascontiguousarray(xs.T),
            "W1": np.asarray(W1, np.float32),
            "W2": np.asarray(W2, np.float32),
            "b1r": np.tile(np.asarray(b1, np.float32)[None, :], (P, 1)),
            "b2r": np.tile(np.asarray(b2, np.float32)[None, :], (P, 1)),
            "iota": _to_bf16(iota_np),
            "dinv": np.ascontiguousarray(dv_pad.reshape(nblk, P).T),
            "gidx": gi,
            "dstrel": _to_bf16(sch.core_dstrel[c]),
        })
    return in_maps


def gcn_reference_np(x, src, dst, W1, b1, W2, b2):
    n = x.shape[0]
    deg = np.bincount(dst, minlength=n).astype(np.float32) + 1.0
    dinv = 1.0 / np.sqrt(deg)

    def conv(h, W, b):
        h = h @ W
        norm = dinv[src] * dinv[dst]
        agg = np.zeros_like(h)
        np.add.at(agg, dst, h[src] * norm[:, None])
        agg = agg + h * (dinv * dinv)[:, None]
        return agg + b

    h = np.maximum(conv(x, W1, b1), 0.0)
    z = conv(h, W2, b2)
    z = z - z.max(axis=1, keepdims=True)
    return z - np.log(np.exp(z).sum(axis=1, keepdims=True))


def _ensure_ntff_hook():
    import types
    try:
        from antenv import axon_hooks  # noqa: F401
        return
    except ImportError:
        pass
    try:
        from trn_agent_boot.trn_boot import _ntff_profile_via_ctypes
        hook = _ntff_profile_via_ctypes("/opt/axon/libaxon_pjrt.so")
        m = types.ModuleType("antenv.axon_hooks")
        m.get_axon_ntff_profile_hook = lambda: hook
        m.set_axon_ntff_profile_hook = lambda h: None
        sys.modules["antenv.axon_hooks"] = m
    except Exception:
        pass


def _to_bf16(a):
    import ml_dtypes
    return a.astype(ml_dtypes.bfloat16)


def kernel(x, edge_index, W1, b1, W2, b2):
    _phases = int(os.environ.get("GCN_PHASES", "5"))
    x = np.asarray(x)
    edge_index = np.asarray(edge_index)
    n_nodes = x.shape[0]
    n_cores = 8
    src = edge_index[0].astype(np.int64)
    dst = edge_index[1].astype(np.int64)

    ck = (n_nodes, edge_index.shape[1],
          int(edge_index[:, :100].sum()), int(edge_index[:, -100:].sum()))
    if ck in _cache:
        sch, nc = _cache[ck]
    else:
        sch = build_schedule(src, dst, n_nodes, n_cores)
        nc = build_program(sch, phases=_phases)
        _cache[ck] = (sch, nc)

    in_maps = make_in_maps(sch, x, dst, W1, b1, W2, b2)

    from concourse.bass_utils import run_bass_kernel_spmd
    trace = bool(int(os.environ.get("GCN_TRACE", "0")))
    if trace:
        _ensure_ntff_hook()
    try:
        res = run_bass_kernel_spmd(nc, in_maps, core_ids=list(range(n_cores)),
                                   trace=trace)
    except Exception:
        if not trace:
            raise
        res = run_bass_kernel_spmd(nc, in_maps, core_ids=list(range(n_cores)),
                                   trace=False)
    kernel._last_results = res
    out = np.concatenate([res.results[c]["z"] for c in range(n_cores)],
                         axis=0)
    return out.astype(np.float32)


if __name__ == "__main__":
    rng = np.random.default_rng(0)
    N, E = 4096, 60000
    src = rng.integers(0, N, E)
    dst = rng.integers(0, N, E)
    sch = build_schedule(src, dst, N, 8)
    print("ncols", sch.ncols, "gidx_cols", sch.gidx_cols, "maxL", sch.maxL)
    print("schedule numpy check rel err:",
          numpy_check_schedule(sch, src, dst, N))

